# revision 1
# baseline (speedup 1.0000x reference)
"""AttentionNCF Trainium2 kernel v3 (SPMD over 8 NeuronCores, data-parallel over B).

Changes vs v1 baseline (106.8us):
  - Score matmuls use 32-column PE strips (tile_position): only each group's
    8 nonzero weight columns (padded to a 32-row quarter window) are loaded,
    and matmuls targeting the 4 different quarters pipeline concurrently in
    the PE array -> measured ~54ns/matmul effective (4x) vs 216ns.
  - All-bf16 (no fp8): formation stays on the fast DVE path (481ns/group).
  - cpT_rep is duplicated: DVE reads one copy, ACT the other (concurrent
    same-address SBUF reads by two engines measurably stall DVE).
  - aw = att*um runs on the idle GpSimd/Pool engine (frees DVE).
  - One PSUM score tile [128,1024] (2 banks) per chunk -> single wide exp.
  - Tail: pad-row correction folded into the denominator matmul (contract
    only 104 real partitions in the tail chunk), reciprocal_approx_fast on
    DVE (no Ln/Exp act-table loads), bf16 bcast matmuls, early e_cT half of
    ps1 keeps the PE HAM-warm through the finale.
  - DMA: host pre-tiles big tensors to [128, chunk, x]; one merged transfer
    each for weT|rated and cand, umT in two halves.
  - su matmuls for early chunks deferred until e_r exists (no PE
    head-of-line stall on the weights/rated transfer).
"""

import sys

import ml_dtypes
import numpy as np

sys.path.insert(0, "/opt/trn_rl_repo")

BF = ml_dtypes.bfloat16

import concourse.bass as bass
import concourse.mybir as mybir
import concourse.tile as tile
from concourse import bacc
from concourse.bass_utils import run_bass_kernel_spmd

F32 = mybir.dt.float32
BF16 = mybir.dt.bfloat16
AF = mybir.ActivationFunctionType
ALU = mybir.AluOpType

B, I, D, E, ATT = 8192, 1000, 1000, 64, 16
D1, D2 = 64, 32
NCORES = 8
BC = B // NCORES  # 1024 batch rows per core
DP = 1024
NT = 8  # i-chunks of 128 (7 full + 1 partial of 104)
IP = 1024
WRX = E + I  # 1064
ICHUNK = [128] * 7 + [104]
NACT = 36  # formation groups on ACT (rest on DVE), of 125


def _chunk_order(t):
    """Within-chunk group emission order: round-robin across quarters so
    consecutive matmuls target different 32-row PE strips."""
    ng = ICHUNK[t] // 8
    qs = [[g for g in range(ng) if g // 4 == q] for q in range(4)]
    out = []
    k = 0
    while any(qs):
        if qs[k % 4]:
            out.append(qs[k % 4].pop(0))
        k += 1
    return out


def _form_sched():
    """Global engine assignment per (chunk, group-in-emission-order)."""
    sched = []
    acc = 0.0
    for t in range(NT):
        for g in _chunk_order(t):
            acc += NACT / 125.0
            if acc >= 1.0:
                acc -= 1.0
                sched.append("A")
            else:
                sched.append("D")
    return sched


FORM_SCHED = _form_sched()


def build_nc():
    nc = bacc.Bacc("TRN2", target_bir_lowering=False)

    def inp(name, shape, dt=F32):
        return nc.dram_tensor(name, shape, dt, kind="ExternalInput")

    cpTrep_d = inp("cpTrep", [128, BC], BF16)
    rpcols_d = inp("rpcols", [128, 125])
    w2q_d = inp("w2q", [128, 16 * 32], BF16)  # strip weights per group
    cpackd = inp("cpack", [128, 328])
    bpackd = inp("bpack", [128, 232], BF16)
    um_d = inp("um", [128, NT * BC], BF16)  # [128, chunk, b] host-tiled
    wr_d = inp("wr", [128, NT * WRX], BF16)  # [128, chunk, weT|ratedT]
    cand_d = inp("cand", [128, NT * BC], BF16)  # [128, chunk, b]
    out_d = nc.dram_tensor("out", [1, BC], F32, kind="ExternalOutput")
    dbg_d = nc.dram_tensor("dbg", [1, BC], F32, kind="ExternalOutput")

    with tile.TileContext(nc) as tc:
        with (
            tc.tile_pool(name="const", bufs=1) as cpool,
            tc.tile_pool(name="inbig", bufs=1) as ipool,
            tc.tile_pool(name="stat", bufs=1) as spool,
            tc.tile_pool(name="hform", bufs=12) as hpool,
            tc.tile_pool(name="att", bufs=5) as apool,
            tc.tile_pool(name="aw", bufs=5) as awpool,
            tc.tile_pool(name="fin", bufs=2) as fpool,
            tc.tile_pool(name="pstmp", bufs=2, space="PSUM") as pstmp,
            tc.tile_pool(name="pssc", bufs=2, space="PSUM") as pssc,
            tc.tile_pool(name="pssu", bufs=1, space="PSUM") as pssu,
        ):
            # ---------------- constants / inputs to SBUF ----------------
            cpD = spool.tile([128, BC], BF16)  # DVE's copy
            nc.sync.dma_start(out=cpD[:], in_=cpTrep_d[:])
            cpA = spool.tile([128, BC], BF16)  # ACT's copy
            nc.sync.dma_start(out=cpA[:], in_=cpTrep_d[:])
            rp_cols = cpool.tile([128, 125], F32)
            nc.sync.dma_start(out=rp_cols[:], in_=rpcols_d[:])
            w2q = cpool.tile([128, 16, 32], BF16)
            nc.sync.dma_start(out=w2q[:], in_=w2q_d[:])
            cpack = cpool.tile([128, 328], F32)
            nc.sync.dma_start(out=cpack[:], in_=cpackd[:])
            ident = cpack[:, 0:128]
            be_c = cpack[0:E, 320:321]
            bm1_c = cpack[0:D1, 322:323]
            bm2_c = cpack[0:D2, 323:324]
            bm3_c = cpack[0:1, 324:325]
            bpack = cpool.tile([128, 232], BF16)
            nc.sync.dma_start(out=bpack[:], in_=bpackd[:])
            onescol = bpack[:, 0:1]
            wm1aT = bpack[0:E, 2:66]
            wm1bT = bpack[0:E, 66:130]
            wm2T = bpack[0:D1, 130:162]
            wm3T = bpack[0:D2, 162:163]
            ones64row = bpack[0:1, 164:228]

            um_sb = ipool.tile([128, NT, BC], BF16)
            nc.sync.dma_start(out=um_sb[:, 0:4, :], in_=um_d[:, 0 : 4 * BC])
            wr = ipool.tile([128, NT, WRX], BF16)
            nc.sync.dma_start(out=wr[:], in_=wr_d[:])
            nc.sync.dma_start(out=um_sb[:, 4:8, :], in_=um_d[:, 4 * BC : 8 * BC])
            cand = ipool.tile([128, NT, BC], BF16)
            nc.sync.dma_start(out=cand[:], in_=cand_d[:])

            e_cT = spool.tile([E, BC], BF16)

            def emit_ecT():
                for h in range(2):
                    sl = slice(512 * h, 512 * (h + 1))
                    ps = pstmp.tile([128, 512], F32, tag="tmp", name=f"psec{h}")
                    for c in range(NT):
                        nc.tensor.matmul(
                            ps[:E, :],
                            wr[:, c, 0:E],
                            cand[:, c, sl],
                            start=(c == 0),
                            stop=(c == NT - 1),
                        )
                    nc.scalar.activation(e_cT[:, sl], ps[:E, :], AF.Identity, bias=be_c[:])

            e_r = spool.tile([128, NT * E], BF16)

            def emit_er_setup():
                e_rT = spool.tile([E, IP], F32)
                nc.vector.memset(e_rT[:, I:IP], 0.0)
                for h, n0, nw in ((0, 0, 500), (1, 500, 500)):
                    ps = pstmp.tile([128, 512], F32, tag="tmp")
                    for c in range(NT):
                        nc.tensor.matmul(
                            ps[:E, :nw],
                            wr[:, c, 0:E],
                            wr[:, c, E + n0 : E + n0 + nw],
                            start=(c == 0),
                            stop=(c == NT - 1),
                        )
                    nc.scalar.activation(e_rT[:, n0 : n0 + nw], ps[:E, :nw], AF.Identity, bias=be_c[:])
                for c in range(NT):
                    ps = pstmp.tile([128, 512], F32, tag="tmp")
                    nc.tensor.transpose(ps[:, :E], e_rT[:, 128 * c : 128 * (c + 1)], ident[:E, :E])
                    nc.vector.tensor_copy(e_r[:, E * c : E * (c + 1)], ps[:, :E])

            # ---------------- main loop over i-chunks ----------------
            su0 = pssu.tile([65, 512], F32)  # rows 0:64 user_emb accum, row 64 denom
            su1 = pssu.tile([65, 512], F32)
            sus = (su0, su1)
            state = [None] * NT
            post = [None] * NT

            gctr = [0]  # global emission counter into FORM_SCHED

            def emit_chunk(t):
                order = _chunk_order(t)
                sc = pssc.tile([128, 1024], F32, tag="sc")
                nq = [0] * 4
                for g in order:
                    nq[g // 4] += 1
                qi = [[0] * 4 for _ in range(2)]
                for g in order:
                    eng = FORM_SCHED[gctr[0]]
                    gctr[0] += 1
                    G = 16 * t + g
                    hT = hpool.tile([128, BC], BF16, tag="h")
                    if eng == "A":
                        nc.scalar.activation(hT[:], cpA[:], AF.Relu, bias=rp_cols[:, G : G + 1])
                    else:
                        nc.vector.tensor_scalar(
                            hT[:], cpD[:], rp_cols[:, G : G + 1], 0.0, ALU.add, ALU.max
                        )
                    q = g // 4
                    for h in range(2):
                        nc.tensor.matmul(
                            sc[32 * q : 32 * q + 32, 512 * h : 512 * (h + 1)],
                            w2q[:, g, :],
                            hT[:, 512 * h : 512 * (h + 1)],
                            start=(qi[h][q] == 0),
                            stop=(qi[h][q] == nq[q] - 1),
                            tile_position=(0, 32 * q),
                            skip_group_check=True,
                        )
                        qi[h][q] += 1
                state[t] = sc

            def emit_exp(t):
                sc = state[t]
                att_t = apool.tile([128, BC], BF16, tag="att")
                aw_t = awpool.tile([128, BC], BF16, tag="aw")
                nc.scalar.activation(att_t[:], sc[:], AF.Exp)
                if t < NT - 2:  # Pool is ~3us/op: fine mid-run, not on the tail path
                    nc.gpsimd.tensor_tensor(aw_t[:], att_t[:], um_sb[:, t, :], ALU.mult)
                else:
                    nc.vector.tensor_mul(aw_t[:], att_t[:], um_sb[:, t, :])
                post[t] = (att_t, aw_t)
                state[t] = None

            def emit_su(t):
                att_t, aw_t = post[t]
                ni = ICHUNK[t]  # contract only real rows: tail pad rows excluded
                for h in range(2):
                    sl = slice(512 * h, 512 * (h + 1))
                    nc.tensor.matmul(
                        sus[h][64:65, :], onescol[0:ni, :], att_t[0:ni, sl],
                        start=(t == 0), stop=(t == NT - 1), skip_group_check=True,
                    )
                    nc.tensor.matmul(
                        sus[h][:64, :], e_r[:, E * t : E * (t + 1)], aw_t[:, sl],
                        start=(t == 0), stop=(t == NT - 1), skip_group_check=True,
                    )
                post[t] = None

            for t in range(NT):
                emit_chunk(t)
                if t >= 1:
                    emit_exp(t - 1)
                if t == 3:
                    emit_er_setup()
                if t == 4:
                    emit_su(0)
                    emit_su(1)
                if t == 5:
                    emit_su(2)
                    emit_su(3)
                    emit_ecT()
                if t == 6:
                    emit_su(4)
                    emit_su(5)
                if t == 7:
                    emit_su(6)
            emit_exp(NT - 1)
            emit_su(NT - 1)

            # ---------------- finale: normalize + MLP ----------------
            # ps1 e_cT half starts right after the last su matmul: useful work
            # that also keeps the PE HAM-warm through the recip/cast window.
            o_sb = fpool.tile([1, BC], F32, tag="o")
            dn = fpool.tile([1, 1024], F32, tag="dn")
            rec = fpool.tile([1, 1024], F32, tag="rec")
            rec16 = fpool.tile([1, 1024], BF16, tag="rec16")
            bcast, u_sb, h1s, h2s, ps1s, ps2s, ps3s = {}, {}, {}, {}, {}, {}, {}
            for h in range(2):
                sl = slice(512 * h, 512 * (h + 1))
                ps1s[h] = pstmp.tile([128, 512], F32, tag="tmp", name=f"ps1_{h}")
                nc.tensor.matmul(ps1s[h][:D1, :], wm1aT, e_cT[:, sl], start=True, stop=False)
            for h in range(2):
                sl = slice(512 * h, 512 * (h + 1))
                nc.scalar.activation(dn[:, sl], sus[h][64:65, :], AF.Identity)
            for h in range(2):
                sl = slice(512 * h, 512 * (h + 1))
                nc.vector.reciprocal_approx_fast(out=rec[:, sl], in_=dn[:, sl])
            nc.scalar.activation(rec16[:], rec[:], AF.Identity)
            nc.sync.dma_start(out=dbg_d[:], in_=dn[:])
            for h in range(2):
                sl = slice(512 * h, 512 * (h + 1))
                psb = pssc.tile([128, 1024], F32, tag="sc", name=f"psb{h}")
                nc.tensor.matmul(psb[:E, 0:512], ones64row, rec16[:, sl], start=True, stop=True)
                bcast[h] = fpool.tile([E, 512], F32, tag=f"bcast{h}", name=f"bcast{h}")
                nc.vector.tensor_copy(bcast[h][:], psb[:E, 0:512])
            for h in range(2):
                u_sb[h] = fpool.tile([E, 512], BF16, tag=f"u{h}", name=f"u{h}")
                nc.vector.tensor_mul(u_sb[h][:], sus[h][:64, :], bcast[h][:])
            for h in range(2):
                nc.tensor.matmul(ps1s[h][:D1, :], wm1bT, u_sb[h][:], start=False, stop=True)
            for h in range(2):
                h1s[h] = fpool.tile([D1, 512], BF16, tag=f"h1{h}", name=f"h1{h}")
                nc.scalar.activation(h1s[h][:], ps1s[h][:D1, :], AF.Relu, bias=bm1_c)
            for h in range(2):
                ps2s[h] = pssc.tile([128, 1024], F32, tag="sc", name=f"ps2_{h}")
                nc.tensor.matmul(ps2s[h][:D2, 0:512], wm2T, h1s[h][:], start=True, stop=True)
            for h in range(2):
                h2s[h] = fpool.tile([D2, 512], BF16, tag=f"h2{h}", name=f"h2{h}")
                nc.scalar.activation(h2s[h][:], ps2s[h][:D2, 0:512], AF.Relu, bias=bm2_c)
            for h in range(2):
                ps3s[h] = pstmp.tile([128, 512], F32, tag="tmp", name=f"ps3_{h}")
                nc.tensor.matmul(ps3s[h][:1, :], wm3T, h2s[h][:], start=True, stop=True)
            for h in range(2):
                sl = slice(512 * h, 512 * (h + 1))
                nc.scalar.activation(o_sb[:, sl], ps3s[h][:1, :], AF.Identity, bias=bm3_c)
                nc.sync.dma_start(out=out_d[:, sl], in_=o_sb[:, sl])

    nc.compile()
    return nc


def host_prep(candidate_items, rated_items, user_matrix, We, be, Wa1, ba1, Wa2,
              ba2, Wm1, bm1, Wm2, bm2, Wm3, bm3):
    f = np.float32
    cand = np.asarray(candidate_items, f)
    rated = np.asarray(rated_items, f)
    um = np.asarray(user_matrix, f)
    We = np.asarray(We, f)
    be = np.asarray(be, f)
    Wa1 = np.asarray(Wa1, f)
    Wa2 = np.asarray(Wa2, f)
    Wm1 = np.asarray(Wm1, f)
    bm1 = np.asarray(bm1, f)
    Wm2 = np.asarray(Wm2, f)
    bm2 = np.asarray(bm2, f)
    Wm3 = np.asarray(Wm3, f)
    bm3 = np.asarray(bm3, f)

    W1c, W1r = Wa1[:, :E], Wa1[:, E:]
    wa2 = Wa2[0]  # [ATT]

    def tile128(mat_t, X):  # [1024 rows, X] -> [128, 8, X]
        return np.ascontiguousarray(mat_t.reshape(NT, 128, X).transpose(1, 0, 2))

    candT = np.zeros((DP, B), BF)
    candT[:D] = cand.T.astype(BF)
    umT = np.zeros((IP, B), BF)
    umT[:I] = um.T.astype(BF)
    wrT = np.zeros((DP, WRX), BF)
    wrT[:D, :E] = We.T.astype(BF)
    wrT[:D, E:] = rated.T.astype(BF)
    wr_h = tile128(wrT, WRX).reshape(128, NT * WRX)

    cp_full = (cand @ (W1c @ We).T + (W1c @ be)).astype(f)  # [B, ATT]

    e_r_h = rated @ We.T + be  # [I, E]
    rp = e_r_h @ W1r.T + ba1  # [I, ATT]
    rp_cols = np.zeros((128, 125), f)
    rp_cols[:] = rp.reshape(125, 8, ATT).transpose(1, 2, 0).reshape(128, 125)

    # strip weights [128, g, 32]: group g writes quarter window g//4,
    # its 8 columns at offset 8*(g%4); rows 16*il+a hold wa2[a]
    w2q = np.zeros((128, 16, 32), BF)
    for g in range(16):
        off = 8 * (g % 4)
        for il in range(8):
            for a in range(ATT):
                w2q[16 * il + a, g, off + il] = wa2[a]

    cpackm = np.zeros((128, 328), f)
    cpackm[:, 0:128] = np.eye(128, dtype=f)
    cpackm[:E, 320] = be
    cpackm[:D1, 322] = bm1
    cpackm[:D2, 323] = bm2
    cpackm[0, 324] = bm3[0]

    bpackm = np.zeros((128, 232), BF)
    bpackm[:, 0] = 1.0  # onescol
    bpackm[:E, 2:66] = Wm1[:, :E].T.astype(BF)
    bpackm[:E, 66:130] = Wm1[:, E:].T.astype(BF)
    bpackm[:D1, 130:162] = Wm2.T.astype(BF)
    bpackm[:D2, 162] = Wm3[0].astype(BF)
    bpackm[0, 164:228] = 1.0  # ones64row

    shared = {
        "wr": wr_h,
        "rpcols": rp_cols,
        "w2q": w2q.reshape(128, 16 * 32),
        "cpack": cpackm,
        "bpack": bpackm,
    }
    in_maps = []
    for k in range(NCORES):
        m = dict(shared)
        m["cand"] = tile128(
            np.ascontiguousarray(candT[:, BC * k : BC * (k + 1)]), BC
        ).reshape(128, NT * BC)
        m["um"] = tile128(
            np.ascontiguousarray(umT[:, BC * k : BC * (k + 1)]), BC
        ).reshape(128, NT * BC)
        cpk = cp_full[BC * k : BC * (k + 1)]  # [BC, ATT]
        m["cpTrep"] = np.ascontiguousarray(cpk.T[np.arange(128) % ATT, :]).astype(BF)
        in_maps.append(m)
    return in_maps


_NC_CACHE = {}


def _get_nc():
    if "nc" not in _NC_CACHE:
        _NC_CACHE["nc"] = build_nc()
    return _NC_CACHE["nc"]


def _install_ntff_hook():
    """Provide antenv.axon_hooks (absent in this image) so trace=True works."""
    import contextlib
    import ctypes
    import types

    if "antenv.axon_hooks" in sys.modules:
        return
    mod = types.ModuleType("antenv.axon_hooks")
    holder = {}
    mod.set_axon_ntff_profile_hook = lambda h: holder.__setitem__("h", h)
    mod.get_axon_ntff_profile_hook = lambda: holder.get("h")
    import antenv

    antenv.axon_hooks = mod
    sys.modules["antenv.axon_hooks"] = mod

    so_path = "/opt/axon/libaxon_pjrt.so"
    lib = ctypes.CDLL(so_path)
    if not hasattr(lib, "axon_start_nrt_profile"):
        return
    lib.axon_start_nrt_profile.argtypes = [ctypes.POINTER(ctypes.c_int64), ctypes.c_size_t]
    lib.axon_start_nrt_profile.restype = ctypes.c_int64
    lib.axon_stop_nrt_profile.argtypes = [ctypes.c_char_p]
    lib.axon_stop_nrt_profile.restype = ctypes.c_int64

    @contextlib.contextmanager
    def _hook(output_dir, device_ids):
        import jax

        jax.devices()
        if device_ids:
            ids = (ctypes.c_int64 * len(device_ids))(*device_ids)
            rc = lib.axon_start_nrt_profile(ids, len(device_ids))
        else:
            rc = lib.axon_start_nrt_profile(None, 0)
        if rc != 0:
            raise RuntimeError(f"axon_start_nrt_profile rc={rc}")
        try:
            yield
        finally:
            n = lib.axon_stop_nrt_profile(str(output_dir).encode())
            print(f"ntff profile: {n} file(s) written to {output_dir}", file=sys.stderr)

    mod.set_axon_ntff_profile_hook(_hook)


def run(inputs, trace=False, **kw):
    if trace:
        _install_ntff_hook()
    nc = _get_nc()
    in_maps = host_prep(**inputs)
    res = run_bass_kernel_spmd(nc, in_maps, list(range(NCORES)), trace=trace, **kw)
    out = np.concatenate(
        [np.asarray(res.results[k]["out"]).reshape(BC, 1) for k in range(NCORES)], axis=0
    ).astype(np.float32)
    run.dbg = np.concatenate(
        [np.asarray(res.results[k]["dbg"]).reshape(BC) for k in range(NCORES)]
    ).astype(np.float32)
    return out, res


def kernel(**inputs):
    out, _ = run(inputs, trace=False)
    return out

